# revision 59
# baseline (speedup 1.0000x reference)
"""AreaAttention Trainium2 kernel.

Data-parallel over batch: 8 batches -> 8 NeuronCores, one batch per core.

The reference's `qk.reshape(B*AREA, 2C, N//AREA)` is a *row-major* reshape:
area a = channel block [128a, 128a+128) of the 512 qk channels (NOT a spatial
split). Within an area, the attention feature mixes channel offset j in
[0,8) with spatial quarter g = n // 1024 (packed here as d'' = 4j + g so
every regroup/scatter DMA keeps its partition stride in AP dim 0, which the
HW DGE requires); tokens are n' = n % 1024.  For v (256 channels): area a =
channels [64a, 64a+64), head h = offset [8h, 8h+8), same (g, n') split.
The attention output lands back at channel 64a+8h+j, spatial 1024g+n'.

Per core pipeline (one software-pipelined emission across all 16
(area, head-pair) tiles; QK always one step ahead of exp/AV so the in-order
PE queue never parks behind an exp dependency):
  1. qk conv (w_qk rows host-permuted to (half, area, head, j) order),
     fp16 matmuls, DVE affine epilogue into a staging tile; one 3-dim
     SBUF->SBUF DMA per (tau, area, head) regroups (j, g) -> 32 contiguous
     partitions (taus 0/2 via Pool SWDGE, taus 1/3 via SP/HWDGE).
  2. vT conv in transposed orientation (x^T @ w_v^T); the epilogue writes
     *directly* into the AV lhsT layout [m'-tile, area, head, d''] with
     s_v prescaled, plus a ones column per head (fused softmax denominator).
  3. v4 conv + depthwise 3x3 via diagonal-matrix matmuls (zero-padded
     fp16 image), affine epilogue with b_pe+b_v folded, written straight
     into t_sb as the pe pre-fill; conv work rides the attention slack as
     filler closures between pairs.
  4. attention per (area, head-pair): T = K^T Q (row-packed K=32 fp16
     matmuls) -> exp on ACT (PSUM->SBUF, fused SCALE, no max-subtraction
     needed) -> O/Z = [V;1]^T @ expT (col-packed M=33 fp16 matmuls
     accumulated per nh half into a single-bank PSUM tile).  Per-half
     epilogue: stage O+Z to SBUF, 1/Z via one Newton step from an integer
     magic seed, broadcast, multiply, and one accumulating SWDGE DMA per
     head adds O/Z onto the pe pre-fill in t.
  5. proj conv (fp16) + affine into the fp16 output (host casts to f32).
"""

import os
import sys

# Defensive: if a previous process wedged the NeuronCores, a reset at
# runtime init recovers them (one-time init cost, no effect on execution).
os.environ.setdefault("NEURON_RT_RESET_CORES", "1")

try:
    import concourse  # noqa: F401
except ImportError:  # pragma: no cover
    sys.path.insert(0, "/opt/trn_rl_repo")

from contextlib import ExitStack

import numpy as np

import concourse.bass as bass
import concourse.mybir as mybir
import concourse.tile as tile
from concourse import bacc

P = 128
C = 256
H = W = 64
N = H * W          # 4096
AREA = 4
NA = N // AREA     # 1024
HEADS = 8
HD = 32
SCALE = float(HD) ** -0.5
B = 8

f32 = mybir.dt.float32
f16 = mybir.dt.float16
i32 = mybir.dt.int32

# u0 = -(magic - Z_bits):  (Z ^ -1) + (MAGIC + 1 + 0x80000000)  (int32 wrap)
_K = (0x7EF127EA + 1 + 0x80000000) & 0xFFFFFFFF
MAGIC_NEG = _K - 0x100000000 if _K >= 0x80000000 else _K

Exp = mybir.ActivationFunctionType.Exp
Ident = mybir.ActivationFunctionType.Identity
MULT = mybir.AluOpType.mult
ADD = mybir.AluOpType.add


def build_module():
    nc = bacc.Bacc("TRN2", target_bir_lowering=False, debug=False)

    # ---- DRAM I/O ------------------------------------------------------
    xb = nc.dram_tensor("xb", [P, 2, N], f16, kind="ExternalInput").ap()
    # packed constants: one f16 blob + one f32 blob -> 2 DMAs, 2 semaphores
    wf16 = nc.dram_tensor("wf16", [P, 4352], f16, kind="ExternalInput").ap()
    cf32 = nc.dram_tensor("cf32", [P, 280], f32, kind="ExternalInput").ap()
    out = nc.dram_tensor("out", [P, 2, N], f16, kind="ExternalOutput").ap()

    with tile.TileContext(nc) as tc, ExitStack() as ctx:
        const = ctx.enter_context(tc.tile_pool(name="const", bufs=1))
        big = ctx.enter_context(tc.tile_pool(name="big", bufs=1))
        work = ctx.enter_context(tc.tile_pool(name="work", bufs=1))

        # ---- constants / weights in SBUF ---------------------------------
        # wqkT + cf32 land first and in their own tiles (they gate the first
        # stage matmul/epilogue); the remaining weights stream in behind.
        wqk_sb = const.tile([P, 1024], f16)
        nc.sync.dma_start(wqk_sb[:], wf16[:, 0:1024])
        cf32_sb = const.tile([P, 280], f32)
        nc.sync.dma_start(cf32_sb[:], cf32)
        wrest_sb = const.tile([P, 3328], f16)
        nc.sync.dma_start(wrest_sb[:], wf16[:, 1024:4352])
        wqkT_sb = wqk_sb[:].rearrange("p (k m) -> p k m", k=2)
        wvT_sb = wrest_sb[:, 0:512].rearrange("p (k m) -> p k m", k=2)
        wpjT_sb = wrest_sb[:, 512:1024].rearrange("p (k m) -> p k m", k=2)
        dw_sb = wrest_sb[:, 1024:3328].rearrange("p (c t m) -> p c t m", c=2, t=9)
        svb_sb = cf32_sb[:, 0:256]
        sqk_sb = cf32_sb[:, 256:260]
        bqk_sb = cf32_sb[:, 260:264]
        sv_sb = cf32_sb[:, 264:266]
        bv_sb = cf32_sb[:, 266:268]
        spe_sb = cf32_sb[:, 268:270]
        bpe2_sb = cf32_sb[:, 270:272]
        spj_sb = cf32_sb[:, 272:274]
        bpj_sb = cf32_sb[:, 274:276]

        # ---- persistent activations --------------------------------------
        # qkr: [p = 32*(h%4) + 8g + j, half(q/k), area, h//4, n']
        qkr = big.tile([P, 2, AREA, 2, NA], f16)
        # vTr: [p = m' % 128, m'-tile jj, area, head, d'=8g+j plus ones col]
        vTr = big.tile([P, 8, AREA, HEADS, 33], f16)
        # t_sb is fully written by the attention scatters; the positional-
        # encoding term lands in pe_sb and is added per ct once that ct's
        # scatters are done (cheaper than ordering dw against accum-DMAs).
        t_sb = big.tile([P, 2, N], f16)
        pe_sb = big.tile([P, 2, N], f16)
        # ones column for the fused-Z trick
        nc.gpsimd.memset(vTr[:, :, :, :, 32:33], 1.0)
        # persistent Newton-reciprocal staging: rows 0 / 32 hold each
        # pair's Z; remaining rows stay 1.0 so batched ops read defined data.
        Zt = big.tile([64, NA], f32)
        nc.gpsimd.memset(Zt[:], 1.0)

        with (
            tc.tile_pool(name="ph1", bufs=1) as ph1,
            tc.tile_pool(name="psc", bufs=2, space="PSUM") as psc,
            tc.tile_pool(name="psa", bufs=1, space="PSUM") as psa,
            tc.tile_pool(name="pst", bufs=2, space="PSUM") as pst,
        ):
            # split the input load so the first qk-stage chunks (cols 0:1024)
            # can start as soon as ~1/4 of the input has landed.
            xb_sb = ph1.tile([P, 2, N], f16)
            nc.sync.dma_start(xb_sb[:, :, 0:1024], xb[:, :, 0:1024])
            v4pad = ph1.tile([P, 2, H + 2, W + 2], f16)
            nc.gpsimd.memset(v4pad[:], 0.0)
            nc.sync.dma_start(xb_sb[:, :, 1024:], xb[:, :, 1024:])

            def vt_conv():
                # ---- vT conv (transposed orientation, s_v prescaled) ----------
                # spatial n = 128t + p with t = 8g + jj  ->  writes the
                # d'' = 4j + g columns of vTr[:, jj, :, :, :] (stride-4
                # slice at offset g).  jj-major order so the AV j=0 lhsT
                # tiles are complete after 4 iterations.
                for jj in range(8):
                  for g in range(4):
                    t = 8 * g + jj
                    cps = psc.tile([P, 512], f32, tag="cps")
                    vtp = cps[:, 0:256]
                    for kt in range(2):
                        nc.tensor.matmul(
                            vtp,
                            xb_sb[:, kt, 128 * t : 128 * (t + 1)],
                            wvT_sb[:, kt, :],
                            start=(kt == 0),
                            stop=(kt == 1),
                        )
                    nc.vector.tensor_tensor(
                        vTr[:, jj, :, :, 0:32].rearrange(
                            "p a h (j gg) -> gg p a h j", gg=4
                        )[g],
                        vtp.rearrange("p (a h j) -> p a h j", a=AREA, h=HEADS),
                        svb_sb[:].rearrange("p (a h j) -> p a h j", a=AREA, h=HEADS),
                        MULT,
                    )

            def qk_stage(tau):
                # conv output channel (host-permuted): c = half*256 + a*64
                # + 8h + j -> tile tau = 2*half + a//2, partition
                # 64*(a%2) + 8h + j.  DVE affine epilogue into a staging tile.
                stage = ph1.tile([P, N], f16, tag="stage", bufs=2, name=f"st{tau}")
                for nchunk in range(8):
                    qk_stage_chunk(tau, stage, nchunk)
                return stage

            def qk_regroup(tauq, stq, stk, eng):
                # one 3-dim DMA per (tau, area, head) regroups (j, g) -> 32
                # contiguous partitions.  The attention feature is packed as
                # d'' = 4j + g (j-major) so BOTH APs keep their partition
                # stride in dim 0 — the HW DGE rejects partition steps in
                # inner dims:
                #   dest qkr[32*(h%4) + 4*j + g, half, a, h//4, n']
                #    <-  stage[64*(a%2) + 8*h + j, 1024*g + n']
                # q/k interleaved per head so the first QK pair unblocks
                # after 4 DMAs instead of a full tau's worth.
                ah = tauq % 2
                for a in (2 * ah, 2 * ah + 1):
                    for h in range(HEADS):
                        rb = 32 * (h % 4)
                        sb = 64 * (a % 2) + 8 * h
                        for half, stage in ((0, stq), (1, stk)):
                            eng.dma_start(
                                qkr[rb : rb + 32, half, a, h // 4, :],
                                stage[sb : sb + 8, :].rearrange(
                                    "j (g n) -> j g n", g=4
                                ),
                            )

            def qk_conv(tau_pair, eng):
                tauq, tauk = tau_pair, tau_pair + 2
                stq = qk_stage(tauq)
                stk = qk_stage(tauk)
                qk_regroup(tauq, stq, stk, eng)

            def qk_stage_chunk(tau, stage, nchunk):
                cps = psc.tile([P, 512], f32, tag="cps", name=f"qk{tau}_{nchunk}")
                for kt in range(2):
                    nc.tensor.matmul(
                        cps[:],
                        wqkT_sb[:, kt, 128 * tau : 128 * (tau + 1)],
                        xb_sb[:, kt, 512 * nchunk : 512 * (nchunk + 1)],
                        start=(kt == 0),
                        stop=(kt == 1),
                    )
                nc.vector.tensor_scalar(
                    stage[:, 512 * nchunk : 512 * (nchunk + 1)],
                    cps[:],
                    sqk_sb[:, tau : tau + 1],
                    bqk_sb[:, tau : tau + 1],
                    MULT,
                    ADD,
                )

            def qk_fillers(tau_pair, eng):
                tauq, tauk = tau_pair, tau_pair + 2
                stq = ph1.tile([P, N], f16, tag="stage", bufs=2, name=f"st{tauq}")
                stk = ph1.tile([P, N], f16, tag="stage", bufs=2, name=f"st{tauk}")
                fs = [
                    (lambda t=tauq, s=stq, c=nchunk: qk_stage_chunk(t, s, c))
                    for nchunk in range(8)
                ] + [
                    (lambda t=tauk, s=stk, c=nchunk: qk_stage_chunk(t, s, c))
                    for nchunk in range(8)
                ]
                fs.append(lambda: qk_regroup(tauq, stq, stk, eng))
                return fs

            def v4_chunk(ct, rc):
                # v4 conv (normal orientation) into the padded image.
                cps = psc.tile([P, 512], f32, tag="cps", name=f"v4_{ct}_{rc}")
                for kt in range(2):
                    nc.tensor.matmul(
                        cps[:],
                        wvT_sb[:, kt, 128 * ct : 128 * (ct + 1)],
                        xb_sb[:, kt, 512 * rc : 512 * (rc + 1)],
                        start=(kt == 0),
                        stop=(kt == 1),
                    )
                nc.vector.tensor_scalar(
                    v4pad[:, ct, 1 + 8 * rc : 1 + 8 * (rc + 1), 1 : 1 + W],
                    cps[:],
                    sv_sb[:, ct : ct + 1],
                    bv_sb[:, ct : ct + 1],
                    MULT,
                    ADD,
                )

            def dw_chunk(ct, rc):
                # depthwise 3x3 via diagonal matmuls; the epilogue writes
                # t_sb directly — attention scatters accumulate on top.
                cps = psc.tile([P, 512], f32, tag="cps", name=f"dw_{ct}_{rc}")
                for tap in range(9):
                    dy, dx = tap // 3 - 1, tap % 3 - 1
                    rhs = v4pad[
                        :,
                        ct,
                        1 + dy + 8 * rc : 1 + dy + 8 * (rc + 1),
                        1 + dx : 1 + dx + W,
                    ]
                    nc.tensor.matmul(
                        cps[:],
                        dw_sb[:, ct, tap, :],
                        rhs,
                        start=(tap == 0),
                        stop=(tap == 8),
                    )
                nc.vector.tensor_scalar(
                    pe_sb[:, ct, 512 * rc : 512 * (rc + 1)],
                    cps[:],
                    spe_sb[:, ct : ct + 1],
                    bpe2_sb[:, ct : ct + 1],
                    MULT,
                    ADD,
                )

            def v4_dw_fillers(ct):
                return [
                    (lambda c=ct, r=rc: v4_chunk(c, r)) for rc in range(8)
                ] + [
                    (lambda c=ct, r=rc: dw_chunk(c, r)) for rc in range(8)
                ]

            # av is split per nh half (one PSUM bank each, double-buffered):
            # the half-epilogue drains av(nh) while the other half's 8 AV
            # steps run, so allocations never stall PE at a pair boundary.
            avs = {}

            def qk_step(a, pr, nh, j):
                hA, hB = 2 * pr, 2 * pr + 1
                Tp = pst.tile([P, NA], f32, tag="Tp",
                              name=f"Tp{a}{pr}{nh}{j}")
                for h, cb in ((hA, 0), (hB, 512)):
                    rb = 32 * (h % 4)
                    lhsT = qkr[
                        rb : rb + 32, 1, a, h // 4,
                        128 * j : 128 * (j + 1),
                    ]
                    rhs = qkr[
                        rb : rb + 32, 0, a, h // 4,
                        512 * nh : 512 * (nh + 1),
                    ]
                    nc.tensor.matmul(
                        Tp[:, cb : cb + 512],
                        lhsT,
                        rhs,
                        start=True,
                        stop=True,
                        tile_position=(rb, 0),
                    )
                return Tp

            def half_epilogue(a, pr, nh):
                # normalize O by 1/Z and scatter back to (c, n) space, one
                # nh half (512 tokens) at a time.  Stage O + Z out of PSUM
                # first (4 DVE copies) so the av bank frees while the other
                # half computes.  1/Z via ONE Newton iteration from an
                # integer magic seed (u0 rel-err ~3.4% -> ~0.1% after one
                # step; far inside the 2e-2 gate).  The chain tracks u = -r
                # so the final minus sign folds into the O multiply.
                av = avs.pop((pr, nh))
                zc = 512 * nh
                Zs = Zt[:, zc : zc + 512]
                nc.vector.tensor_copy(Zs[0:1, :], av[32:33, :])
                nc.vector.tensor_copy(Zs[32:33, :], av[96:97, :])
                Ob = work.tile([64, 512], f16, tag="Ob", bufs=2,
                               name=f"Ob{a}_{pr}_{nh}")
                nc.vector.tensor_copy(Ob[0:32, :], av[0:32, :])
                nc.vector.tensor_copy(Ob[32:64, :], av[64:96, :])
                u0 = work.tile([64, 512], f32, tag="uu", bufs=2,
                               name=f"u0{a}_{pr}_{nh}")
                nc.vector.tensor_scalar(
                    u0[:].bitcast(i32), Zs.bitcast(i32),
                    -1, None, mybir.AluOpType.bitwise_xor,
                )
                nc.vector.tensor_scalar(
                    u0[:].bitcast(i32), u0[:].bitcast(i32),
                    MAGIC_NEG, None, ADD,
                )
                e0 = work.tile([64, 512], f32, tag="ee", bufs=2,
                               name=f"e0{a}_{pr}_{nh}")
                nc.vector.tensor_tensor(e0[:], Zs, u0[:], MULT)
                u1 = work.tile([64, 512], f32, tag="uu", bufs=2,
                               name=f"u1{a}_{pr}_{nh}")
                nc.vector.scalar_tensor_tensor(
                    u1[:], e0[:], 2.0, u0[:], ADD, MULT
                )
                rZb = work.tile([64, 512], f32, tag="rZb", bufs=2,
                                name=f"rZb{a}_{pr}_{nh}")
                nc.vector.stream_shuffle(rZb[:], u1[:], [0] * 32)
                tmp16 = work.tile([64, 512], f16, tag="tmp16", bufs=8,
                                  name=f"tm{a}{pr}{nh}")
                nc.vector.scalar_tensor_tensor(
                    tmp16[0:32, :], Ob[0:32, :], -1.0, rZb[0:32, :],
                    MULT, MULT,
                )
                nc.vector.scalar_tensor_tensor(
                    tmp16[32:64, :], Ob[32:64, :], -1.0, rZb[32:64, :],
                    MULT, MULT,
                )
                # one 3-dim accumulating DMA per head scatters this half's
                # 4 g-quarters onto the pre-filled pe term (tmp16 partitions
                # are d''-ordered = 4j + g, so src iterates (j, g, n) with a
                # plain ascending partition dim — HW-legal on both sides):
                #   t_sb[64*(a%2) + 8*h + j, a//2, 1024*g + 512*nh + n'']
                #     += tmp16[32*h2 + 4*j + g, n'']
                for h2 in range(2):
                    db = 64 * (a % 2) + 8 * (2 * pr + h2)
                    nc.sync.dma_start(
                        t_sb[db : db + 8, a // 2, :].rearrange(
                            "j (g m n) -> m j g n", g=4, m=2
                        )[nh],
                        tmp16[32 * h2 : 32 * h2 + 32, :],
                    )

            def av_step(a, pr, nh, j, Tp):
                hA, hB = 2 * pr, 2 * pr + 1
                if j == 0:
                    avs[(pr, nh)] = psa.tile([P, 512], f32, tag="av",
                                             bufs=2, name=f"av{a}_{pr}_{nh}")
                av = avs[(pr, nh)]
                expT = work.tile([P, NA], f16, tag="expT", bufs=6,
                                 name=f"e{a}{pr}{nh}{j}")
                nc.scalar.activation(expT[:], Tp[:], Exp, scale=SCALE)
                for h, cb, pb in ((hA, 0, 0), (hB, 512, 64)):
                    vaug = vTr[:, j, a, h, :]
                    nc.tensor.matmul(
                        av[pb : pb + 33, :],
                        vaug,
                        expT[:, cb : cb + 512],
                        start=(j == 0),
                        stop=(j == 7),
                        tile_position=(0, pb),
                        skip_group_check=True,
                    )
                if j == 7:
                    half_epilogue(a, pr, nh)

            def attention_all(fillers_by_pair):
                # One software pipeline across ALL areas and pairs: QK is
                # emitted one step ahead of exp/AV so the in-order PE queue
                # never parks AV(k) (waiting on exp(k)) in front of QK(k+1)
                # — not even across pair/area boundaries.  Conv-work filler
                # closures are injected at pair-end emission points so the
                # scheduler interleaves them into attention's slack.
                steps = [(a, pr, nh, j)
                         for a in range(4) for pr in range(4)
                         for nh in range(2) for j in range(8)]
                prev = None
                for s in steps:
                    Tp = qk_step(*s)
                    if prev is not None:
                        av_step(*prev)
                        pa, ppr, pnh, pj, _ = prev
                        if pnh == 1 and pj == 7:
                            for f in fillers_by_pair.get((pa, ppr), ()):
                                f()
                    prev = (*s, Tp)
                av_step(*prev)
                pa, ppr = prev[0], prev[1]
                for f in fillers_by_pair.get((pa, ppr), ()):
                    f()

            # areas 0/1 need qk taus 0 (q) and 2 (k); areas 2/3 need taus
            # 1/3.  With pe in its own tile there is no ordering hazard
            # against the scatters, so all conv work rides attention slack:
            # dw-ct0 in area 0, the tau1/3 qk stages in area 1, dw-ct1 in
            # area 2.  pe-add(ct0) fires at area 1's end (Pool, hidden);
            # pe-add(ct1) is split by nh column parity so only the odd half
            # sits on the tail, matching proj's even-first order.
            def pe_add0():
                nc.gpsimd.tensor_tensor(
                    t_sb[:, 0, :], t_sb[:, 0, :], pe_sb[:, 0, :], ADD
                )

            def pe_add1(m):
                tv = t_sb[:, 1, :].rearrange("p (g m n) -> m p g n", g=4, m=2)[m]
                pv = pe_sb[:, 1, :].rearrange("p (g m n) -> m p g n", g=4, m=2)[m]
                nc.vector.tensor_tensor(tv, tv, pv, ADD)

            qk_conv(0, nc.sync)
            vt_conv()
            fb = {}
            vdw0 = v4_dw_fillers(0)
            for pr in range(4):
                fb[(0, pr)] = vdw0[4 * pr : 4 * pr + 4]
            qf = qk_fillers(1, nc.sync)
            for pr, sl in enumerate((qf[0:6], qf[6:12], qf[12:17], [pe_add0])):
                fb[(1, pr)] = sl
            vdw1 = v4_dw_fillers(1)
            for pr in range(4):
                fb[(2, pr)] = vdw1[4 * pr : 4 * pr + 4]
            attention_all(fb)
            pe_add1(0)
            pe_add1(1)

        # ---- proj conv (needs all areas' t) -----------------------------
        # Column block nq only needs the nh = nq%2 half of every pair's
        # scatter, so even-nh blocks run while the last nh=1 half-epilogue
        # is still in flight.  The affine epilogue rides the ACT engine
        # (idle once the exps are done), keeping DVE off the tail path.
        with tc.tile_pool(name="psp", bufs=4, space="PSUM") as psp:
            for nq in (0, 2, 4, 6, 1, 3, 5, 7):
                for ct in range(2):
                    pj = psp.tile([P, 512], f32, tag="pj")
                    for kt in range(2):
                        nc.tensor.matmul(
                            pj[:],
                            wpjT_sb[:, kt, 128 * ct : 128 * (ct + 1)],
                            t_sb[:, kt, 512 * nq : 512 * (nq + 1)],
                            start=(kt == 0),
                            stop=(kt == 1),
                        )
                    outc = work.tile([P, 512], f16, tag="outc", bufs=4)
                    nc.scalar.activation(
                        outc[:],
                        pj[:],
                        Ident,
                        bias=bpj_sb[:, ct : ct + 1],
                        scale=spj_sb[:, ct : ct + 1],
                    )
                    nc.sync.dma_start(
                        out[:, ct, 512 * nq : 512 * (nq + 1)], outc[:]
                    )
    nc.compile()
    return nc


def make_in_maps(x, w_qk, s_qk, b_qk, w_v, s_v, b_v, w_pe, s_pe, b_pe,
                 w_proj, s_proj, b_proj):
    """Host-side sharding + layout prep. Returns list of 8 per-core dicts."""
    x = np.asarray(x, dtype=np.float32)
    w_qk = np.asarray(w_qk, dtype=np.float32)
    w_v = np.asarray(w_v, dtype=np.float32)
    w_pe = np.asarray(w_pe, dtype=np.float32)
    w_proj = np.asarray(w_proj, dtype=np.float32)
    s_qk, b_qk = np.asarray(s_qk, np.float32), np.asarray(b_qk, np.float32)
    s_v, b_v = np.asarray(s_v, np.float32), np.asarray(b_v, np.float32)
    s_pe, b_pe = np.asarray(s_pe, np.float32), np.asarray(b_pe, np.float32)
    s_proj, b_proj = np.asarray(s_proj, np.float32), np.asarray(b_proj, np.float32)

    # permute qk conv outputs: new channel (half, a, h, j) <- 128a+64*half+8h+j
    perm = np.empty(2 * C, np.int64)
    i = 0
    for half in range(2):
        for a in range(AREA):
            for h in range(HEADS):
                for j in range(8):
                    perm[i] = 128 * a + 64 * half + 8 * h + j
                    i += 1
    w_qk_p, s_qk_p, b_qk_p = w_qk[perm], s_qk[perm], b_qk[perm]

    def ptile2(w):  # [256, M] -> [128, 2, M]
        k, m = w.shape
        return np.ascontiguousarray(w.reshape(2, P, m).transpose(1, 0, 2))

    wpe9 = w_pe.reshape(C, 9).astype(np.float16)
    dwd = np.zeros((P, 2, 9, P), np.float16)
    for ct in range(2):
        for tap in range(9):
            np.fill_diagonal(dwd[:, ct, tap, :], wpe9[128 * ct : 128 * (ct + 1), tap])

    wf16 = np.concatenate(
        [
            ptile2(w_qk_p.T).astype(np.float16).reshape(P, -1),
            ptile2(w_v.T).astype(np.float16).reshape(P, -1),
            ptile2(w_proj.T).astype(np.float16).reshape(P, -1),
            dwd.reshape(P, -1),
        ],
        axis=1,
    )
    cf32 = np.concatenate(
        [
            np.broadcast_to(s_v.reshape(1, C), (P, C)),
            s_qk_p.reshape(4, P).T,
            b_qk_p.reshape(4, P).T,
            s_v.reshape(2, P).T,
            b_v.reshape(2, P).T,
            s_pe.reshape(2, P).T,
            (b_pe + b_v).reshape(2, P).T,
            s_proj.reshape(2, P).T,
            b_proj.reshape(2, P).T,
            np.zeros((P, 4), np.float32),
        ],
        axis=1,
    ).astype(np.float32)
    shared = {
        "wf16": np.ascontiguousarray(wf16),
        "cf32": np.ascontiguousarray(cf32),
    }
    in_maps = []
    for b in range(B):
        m = dict(shared)
        m["xb"] = np.ascontiguousarray(
            x[b].reshape(2, P, N).transpose(1, 0, 2)
        ).astype(np.float16)
        in_maps.append(m)
    return in_maps


def assemble_output(results):
    """results: list of 8 per-core dicts with 'out' [128, 2, N] fp16."""
    arr = np.stack([results[b]["out"] for b in range(B)])  # [B, 128, 2, N]
    return np.ascontiguousarray(arr.transpose(0, 2, 1, 3)).reshape(
        B, C, H, W
    ).astype(np.float32)


_NC = None


def get_module():
    global _NC
    if _NC is None:
        _NC = build_module()
    return _NC


def kernel(**inputs) -> np.ndarray:
    from concourse.bass_utils import run_bass_kernel_spmd

    nc = get_module()
    in_maps = make_in_maps(**inputs)
    res = run_bass_kernel_spmd(nc, in_maps, core_ids=list(range(B)))
    return assemble_output(res.results)

